# revision 19
# baseline (speedup 1.0000x reference)
"""GCN layer (gather + segment-sum + degree-normalize + linear) on 8 Trainium2 cores.

Strategy
--------
Destination-node sharding: core k owns dest rows [k*D, (k+1)*D), D = n_nodes/8.
The host groups each core's edges by 128-dest windows (dest-sorted); the
on-device segment-sum is done per 128-edge chunk with a PE matmul
(lhsT = gathered source features G [128 edge, 128 feat] bf16, rhs = selection
matrix S [128 edge, 128 dest] fp8 with S[e, j] = (col_rel[e] == j)),
accumulating aggT[feat, dest] in PSUM per window. aggT is copied to SBUF
(Scalar) and used directly as lhsT of the linear matmul; bias is folded into
that matmul's PSUM accumulation as a K=1 matmul of deg[j] x bias[f], so the
Scalar PSUM->SBUF copy with a per-partition 1/max(deg,1) scale yields
h @ W.T + b exactly with no DVE add. S is built on DVE in 32-chunk batches
(bf16 crel/iota inputs, fp8 output). No collectives; each core writes its
own output slice and the host concatenates.

The gather stream is the critical path and is DESC-GEN-BOUND: SWDGE Q7
ucode generates ~1 descriptor per 8.0ns per queue solo, degrading to
~9.1ns/desc under SBUF-port contention (G-tile DMA writes, DVE S-builds,
PE reads share ports with the Q7s' idx reads + ring writes); 4 queues run
concurrently for ~2.3ns/desc aggregate over ~108.5K descriptors/core.
Descriptor execution on the 16 DMA engines is NOT the wall (engines ~45%
busy). Every contention cut matters: fp8 S halves the S-build write and PE
read traffic (S is 0/1, exact in fp8; mixed bf16 lhsT x fp8 rhs matmul is
accepted), and the bias fold removes 49 DVE adds.

Schedule: one dma_gather call per 16-chunk (2048-edge) G tile, 16 tiles in
flight per stream; call->queue assignment greedily balances per-queue
descriptor totals (the stream ends when the most-loaded queue finishes) with
a local swap pass; the stream tail is split into 8/8/6/4-chunk calls so the
last wave's post-stream matmul drain is short. gidx loads are split into
2-call (~131KB) slices on the Sync HWDGE queue -- big enough to stream at
full HWDGE rate (small transfers are ~600ns dispatch-bound), small enough
that a call gates only on its own slice; all slices land in the first ~25us.
No warm-up gathers: the ~12us cold start is the one-time global
extended-ucode LOAD_LIB, paid before the first call regardless (first real
gather dispatches at ~17.5us; per-queue first-call cost after the lib load
is ~100ns).

dma_gather facts (measured): idx arrays are int16, wrapped [16, N/16] and
replicated into all eight 16-partition groups; single_packet=False is
required for calls over 1024 indices. int16 limits a gather call's index
range to 32768 rows, so edges are split into lo/hi source streams gathered
from base x[0] / x[32768]. x is gathered in bf16 (256B/row, the descriptor
minimum); rel err ~2.5e-3, well inside the 2e-2 gate. Do NOT mark pad
slots with negative indices to skip their descriptors: the NX-side ring
reservation sizes from num_idxs_reg while the Q7 trims by scanning, and the
mismatch corrupts the descriptor rings (hangs) unless the register carries
the exact per-core post-trim count.

Measured: 280us vs 298us for the 2-lane 32-chunk-block baseline; floor for
this structure is ~247us (solo-rate desc-gen 219us + 17.5us lib load +
~10us drain).
"""
import sys
sys.path.insert(0, "/opt/trn_rl_repo")

import numpy as np

P = 128
GATHER_SPLIT = 32768       # max rows addressable by a signed-int16 gather index
CALL_CHUNKS = 16           # standard gather call / G tile size in 128-edge chunks
SLICE_CALLS = 2            # gidx calls per DMA slice (~131KB each)
SBATCH = 32                # S-matrix build batch, in chunks
GBUFS = 16                 # in-flight G tiles per stream (~4 per queue)
N_CORES = 8


def _ceil_div(a, b):
    return -(-a // b)


def _wrap_idx(ix):
    """[N] int16 -> [128, N/16], idx i at [i%16, i//16], replicated into the
    eight 16-partition groups (the tx/rx Q7 cpus of every SWDGE queue each
    read their own group)."""
    n = len(ix)
    assert n % 16 == 0
    w = np.zeros((P, n // 16), np.int16)
    blk = ix.reshape(-1, 16).T
    for g in range(8):
        w[16 * g:16 * (g + 1), :] = blk
    return w


class Plan:
    """Host-side sharding: per-core per-stream edge arrays with a chunk
    structure (windows x chunk counts) identical across cores, so a single
    SPMD program serves all cores."""

    def __init__(self, row, col, n_nodes, n_cores=N_CORES,
                 call_chunks=CALL_CHUNKS, gather_split=GATHER_SPLIT):
        assert n_nodes % n_cores == 0
        self.n_cores = n_cores
        self.n_nodes = n_nodes
        self.d_core = n_nodes // n_cores
        self.n_win = _ceil_div(self.d_core, P)
        self.call_chunks = call_chunks
        self.gather_split = gather_split

        order = np.argsort(col, kind="stable")
        rs = row[order]
        cs = col[order]
        bounds = np.searchsorted(cs, np.arange(n_cores + 1) * self.d_core)

        # in-degree (clamped to 1) per node, laid out per core as
        # [P, n_win] f32 reciprocal: recip[j, w] = 1/deg of dest w*128+j
        deg = np.bincount(cs, minlength=n_nodes).astype(np.float32)
        deg = np.maximum(deg, 1.0)
        recip = (1.0 / deg)
        pad = self.n_win * P - self.d_core
        self.core_recip = []
        self.core_deg = []
        import ml_dtypes
        for k in range(n_cores):
            r = recip[k * self.d_core:(k + 1) * self.d_core]
            r = np.concatenate([r, np.zeros(pad, np.float32)])
            self.core_recip.append(
                np.ascontiguousarray(r.reshape(self.n_win, P).T))
            dg = deg[k * self.d_core:(k + 1) * self.d_core]
            dg = np.concatenate([dg, np.zeros(pad, np.float32)])
            self.core_deg.append(
                dg[None, :].astype(ml_dtypes.bfloat16))

        W = self.n_win
        per_core = []  # [k][stream] = (rows, local_cols, per-window counts)
        cnt = {"lo": np.zeros(W, np.int64), "hi": np.zeros(W, np.int64)}
        for k in range(n_cores):
            a, b = bounds[k], bounds[k + 1]
            r_k = rs[a:b]
            lc_k = cs[a:b] - k * self.d_core
            lo = r_k < gather_split
            streams = {}
            for sname, mask in (("lo", lo), ("hi", ~lo)):
                r_s = r_k[mask]
                lc_s = lc_k[mask]
                counts = np.bincount(lc_s >> 7, minlength=W)
                streams[sname] = (r_s, lc_s, counts)
                cnt[sname] = np.maximum(cnt[sname], -(-counts // P))
            per_core.append(streams)
        cnt["lo"] = np.maximum(cnt["lo"], 1)  # every window gets >=1 chunk
        self.cnt = cnt
        self.off = {s: np.concatenate([[0], np.cumsum(cnt[s])]) for s in cnt}
        self.Csum = {s: int(self.off[s][-1]) for s in cnt}
        # call partition per stream: standard call_chunks-sized calls, with
        # the tail split into small calls (the last wave's calls land nearly
        # together, so the post-stream matmul drain is one tail-call deep)
        self.call_bounds = {}
        for s in cnt:
            c = self.Csum[s]
            bounds = [0]
            tail = [4, 6, 8, 8]  # last calls, smallest last
            while c - bounds[-1] > sum(tail):
                bounds.append(bounds[-1] + call_chunks)
            rem = c - bounds[-1]
            for t in reversed(tail):
                if rem <= 0:
                    break
                take = min(t, rem)
                bounds.append(bounds[-1] + take)
                rem -= take
            self.call_bounds[s] = np.array(bounds)
        self.NB = {s: len(self.call_bounds[s]) - 1 for s in cnt}
        self.Npad = {s: _ceil_div(self.Csum[s], call_chunks)
                     * call_chunks * P for s in cnt}

        self.core_arrays = []
        import ml_dtypes
        for k in range(n_cores):
            arrs = {}
            for sname in ("lo", "hi"):
                r_s, lc_s, counts = per_core[k][sname]
                off = self.off[sname]
                base = 0 if sname == "lo" else gather_split
                gidx = np.zeros(self.Npad[sname], np.int16)
                crel = np.full(self.Csum[sname] * P, -1, np.int8)
                if len(r_s):
                    starts = np.concatenate([[0], np.cumsum(counts)])
                    adj = off[:-1] * P - starts[:-1]
                    dst = np.arange(len(r_s)) + adj[lc_s >> 7]
                    gidx[dst] = (r_s - base).astype(np.int16)
                    crel[dst] = (lc_s & 127).astype(np.int8)
                arrs[f"gidx_{sname}"] = _wrap_idx(gidx)
                # [P edge-slot, Csum chunk] bf16 (values -1..127, exact)
                arrs[f"crel_{sname}"] = np.ascontiguousarray(
                    crel.reshape(self.Csum[sname], P).T.astype(np.float32)
                ).astype(ml_dtypes.bfloat16)
            self.core_arrays.append(arrs)

    @property
    def total_chunks(self):
        return self.Csum["lo"] + self.Csum["hi"]


def _patch_swdge_lane_by_queue():
    """Pin each dma_gather's DMASW semaphore lane to its SWDGE queue number.

    Tile assigns DMASW lanes round-robin in scheduled order, which breaks when
    instructions on different queues (whose completions are only FIFO within a
    queue) share a lane. Two lanes per queue keep per-lane completion in-order
    and let a call's desc-gen overlap the previous same-queue call's DMA
    completion.
    """
    import concourse.tile_sem_assignment as tsa
    from concourse import mybir
    if getattr(tsa.TileClockTick, "_lane_by_queue_patch", False):
        return
    orig = tsa.TileClockTick._assign_tick

    def patched(self, inst):
        if isinstance(inst, mybir.InstDMAGatherAnt):
            if not hasattr(self, "_q_lane_ctr"):
                self._q_lane_ctr = {}
            q = inst.queue_num
            n = self._q_lane_ctr.get(q, 0)
            self._q_lane_ctr[q] = n + 1
            saved = self.next_sw_dma_idx
            self.next_sw_dma_idx = q * 2 + (n % 2)
            try:
                return orig(self, inst)
            finally:
                self.next_sw_dma_idx = saved
        return orig(self, inst)

    tsa.TileClockTick._assign_tick = patched
    tsa.TileClockTick._lane_by_queue_patch = True


def build_program(plan, in_f, out_f):
    """Emit the SPMD Bass program (shared by all cores)."""
    from concourse import bacc, mybir
    import concourse.tile as tile
    from contextlib import ExitStack

    _patch_swdge_lane_by_queue()

    f32 = mybir.dt.float32
    i16 = mybir.dt.int16
    bf16 = mybir.dt.bfloat16
    fp8 = mybir.dt.float8e4

    W = plan.n_win
    CC = plan.call_chunks

    nc = bacc.Bacc("TRN2", target_bir_lowering=False, debug=False,
                   num_devices=plan.n_cores, num_swdge_queues=4)

    x_d = nc.dram_tensor("xb", [plan.n_nodes, in_f], bf16,
                         kind="ExternalInput")
    wt_d = nc.dram_tensor("wt", [in_f, out_f], bf16, kind="ExternalInput")
    bias_d = nc.dram_tensor("bias", [1, out_f], bf16, kind="ExternalInput")
    deg_d = nc.dram_tensor("deg", [1, W * P], bf16, kind="ExternalInput")
    iota_d = nc.dram_tensor("iota", [P, P], bf16, kind="ExternalInput")
    recip_d = nc.dram_tensor("recip", [P, W], f32, kind="ExternalInput")
    gidx_d, crel_d = {}, {}
    for s in ("lo", "hi"):
        if plan.Csum[s] == 0:
            continue
        gidx_d[s] = nc.dram_tensor(f"gidx_{s}", [P, plan.Npad[s] // 16], i16,
                                   kind="ExternalInput")
        crel_d[s] = nc.dram_tensor(f"crel_{s}", [P, plan.Csum[s]], bf16,
                                   kind="ExternalInput")
    out_d = nc.dram_tensor("out", [W * P, out_f], f32, kind="ExternalOutput")

    x_base = {"lo": x_d[:], "hi": x_d[plan.gather_split:, :]}

    # gather calls in consumption order, interleaved by stream progress;
    # greedy min-load queue assignment (in descriptors) so all 4 queues'
    # desc-gen streams finish together
    def _frac(s, b):
        cb = plan.call_bounds[s]
        return (cb[b] + cb[b + 1]) / 2 / plan.Csum[s]
    call_order = sorted(
        [(s, b) for s in ("lo", "hi") for b in range(plan.NB[s])],
        key=lambda sb: _frac(*sb))
    def _size(sb):
        s, b = sb
        return int(plan.call_bounds[s][b + 1] - plan.call_bounds[s][b])

    qload = [0, 0, 0, 0]
    call_queue = {}
    for s, b in call_order:
        q = min(range(4), key=lambda i: (qload[i], i))
        call_queue[(s, b)] = q
        qload[q] += _size((s, b))
    # local swap pass: equalize per-queue totals (gen-rate-bound stream ends
    # when the most-loaded queue finishes) without moving calls far from
    # their consumption position
    for _ in range(64):
        hi_q = max(range(4), key=lambda i: qload[i])
        lo_q = min(range(4), key=lambda i: qload[i])
        gap = qload[hi_q] - qload[lo_q]
        if gap < 4:
            break
        best = None
        for i, sb in enumerate(call_order):
            if call_queue[sb] != hi_q:
                continue
            for j in range(max(0, i - 4), min(len(call_order), i + 5)):
                sb2 = call_order[j]
                if call_queue[sb2] != lo_q:
                    continue
                d = _size(sb) - _size(sb2)
                if 0 < d <= gap:
                    if best is None or d > best[0]:
                        best = (d, sb, sb2)
        if best is None:
            break
        _, sb, sb2 = best
        call_queue[sb], call_queue[sb2] = lo_q, hi_q
        qload[hi_q] -= best[0]
        qload[lo_q] += best[0]

    with tile.TileContext(nc) as tc, ExitStack() as ctx:
        cpool = ctx.enter_context(tc.tile_pool(name="const", bufs=1))
        gpool = {s: ctx.enter_context(tc.tile_pool(name=f"g_{s}", bufs=GBUFS))
                 for s in ("lo", "hi")}
        spool = {s: ctx.enter_context(tc.tile_pool(name=f"s_{s}", bufs=3))
                 for s in ("lo", "hi")}
        epool = ctx.enter_context(tc.tile_pool(name="epi", bufs=3))
        apool = ctx.enter_context(tc.tile_pool(name="psum_a", bufs=4,
                                               space="PSUM"))
        hpool = ctx.enter_context(tc.tile_pool(name="psum_h", bufs=2,
                                               space="PSUM"))

        # ---- gidx slices: SLICE_CALLS calls each (~131KB), interleaved
        # lo/hi so both streams' first calls gate only on the first slices.
        # All on the Sync HWDGE queue, emitted upfront: big transfers stream
        # at full rate and land before the gather stream needs the engines.
        # slice j covers calls [j*SLICE_CALLS, (j+1)*SLICE_CALLS)
        idx_slices = {}
        slice_bounds = {}
        for s in ("lo", "hi"):
            cb = plan.call_bounds[s]
            sb_ = [int(cb[min(j, plan.NB[s])])
                   for j in range(0, plan.NB[s] + SLICE_CALLS, SLICE_CALLS)]
            slice_bounds[s] = sb_
        slice_order = sorted(
            [(s, j) for s in ("lo", "hi")
             for j in range(len(slice_bounds[s]) - 1)],
            key=lambda sj:
            (slice_bounds[sj[0]][sj[1]] + slice_bounds[sj[0]][sj[1] + 1])
            / 2 / plan.Csum[sj[0]])
        for s, j in slice_order:
            a = slice_bounds[s][j] * P // 16
            z = slice_bounds[s][j + 1] * P // 16
            if z == a:
                continue
            it = cpool.tile([P, z - a], i16, name=f"I{s}{j}")
            nc.sync.dma_start(out=it[:], in_=gidx_d[s][:, a:z])
            idx_slices[(s, j)] = it

        # ---- constants (gate only S builds and the per-window epilogue) ----
        iota_t = cpool.tile([P, P], bf16)
        nc.scalar.dma_start(out=iota_t[:], in_=iota_d[:])
        crel_t = {}
        for s in ("lo", "hi"):
            if plan.Csum[s] == 0:
                continue
            cri = cpool.tile([P, plan.Csum[s]], bf16, name=f"crel{s}")
            nc.scalar.dma_start(out=cri[:], in_=crel_d[s][:])
            crel_t[s] = cri
        wt_t = cpool.tile([in_f, out_f], bf16)
        nc.scalar.dma_start(out=wt_t[:], in_=wt_d[:])
        bias_t = cpool.tile([1, out_f], bf16)
        nc.scalar.dma_start(out=bias_t[:], in_=bias_d[:])
        deg_t = cpool.tile([1, W * P], bf16)
        nc.scalar.dma_start(out=deg_t[:], in_=deg_d[:])
        recip_t = cpool.tile([P, W], f32)
        nc.scalar.dma_start(out=recip_t[:], in_=recip_d[:])

        # ---- gather calls: one per 16-chunk G tile ----
        g_tiles = {}

        def get_g(s, b):
            if (s, b) not in g_tiles:
                cb = plan.call_bounds[s]
                c0, c1 = int(cb[b]), int(cb[b + 1])
                nch = c1 - c0
                gt = gpool[s].tile([P, CC * in_f], bf16, name=f"G{s}{b}",
                                   tag=f"G{s}")
                j = b // SLICE_CALLS
                it = idx_slices[(s, j)]
                off = (c0 - slice_bounds[s][j]) * P // 16
                nc.gpsimd.dma_gather(
                    gt[:, :nch * in_f]
                    .rearrange("p (c e) -> p c e", e=in_f),
                    x_base[s],
                    it[:, off:off + nch * P // 16],
                    nch * P,
                    nch * P,
                    in_f,
                    single_packet=False,
                    queue_num=call_queue[(s, b)],
                )
                g_tiles[(s, b)] = gt
            return g_tiles[(s, b)]

        # ---- lazily-emitted batched S builds (one DVE tensor_tensor
        # is_equal per SBATCH chunks, bf16 in/out) ----
        s_tiles = {}

        def get_s(s, sb):
            if (s, sb) not in s_tiles:
                st = spool[s].tile([P, SBATCH * P], fp8, name=f"S{s}{sb}",
                                   tag=f"S{s}")
                nb = min(SBATCH, plan.Csum[s] - sb * SBATCH)
                in0 = crel_t[s][:, sb * SBATCH:sb * SBATCH + nb] \
                    .to_broadcast([P, nb, P])
                in1 = iota_t[:][:, None, :].to_broadcast([P, nb, P])
                outv = st[:].rearrange("p (b j) -> p b j", j=P)[:, :nb, :]
                nc.vector.tensor_tensor(out=outv, in0=in0, in1=in1,
                                        op=mybir.AluOpType.is_equal)
                s_tiles[(s, sb)] = st
            return s_tiles[(s, sb)]

        # ---- pre-issue every gather call so the Pool engine always has
        # ready calls on all 4 queues ----
        for s, b in call_order:
            get_g(s, b)

        # ---- main window loop ----
        for w in range(W):
            chunks = []
            for s in ("lo", "hi"):
                chunks += [(s, c) for c in
                           range(plan.off[s][w], plan.off[s][w + 1])]
            psum_aggT = apool.tile([P, in_f], f32, tag="aggT",
                                   name=f"aggT{w}")
            n = len(chunks)
            for i, (s, c) in enumerate(chunks):
                b = int(np.searchsorted(plan.call_bounds[s], c,
                                        side="right")) - 1
                slot = c - int(plan.call_bounds[s][b])
                sb, ssub = divmod(c, SBATCH)
                gt = get_g(s, b)
                st = get_s(s, sb)
                nc.tensor.matmul(
                    out=psum_aggT[:],
                    lhsT=gt[:, slot * in_f:(slot + 1) * in_f],
                    rhs=st[:, ssub * P:(ssub + 1) * P],
                    start=(i == 0), stop=(i == n - 1))

            hT_t = epool.tile([P, P], bf16, tag="hT", name=f"hT{w}")
            nc.scalar.activation(out=hT_t[:], in_=psum_aggT[:],
                                 func=mybir.ActivationFunctionType.Copy)
            out_p = hpool.tile([P, out_f], f32, tag="outp", name=f"outp{w}")
            nc.tensor.matmul(out=out_p[:], lhsT=hT_t[:], rhs=wt_t[:],
                             start=True, stop=False)
            # bias folded into the PSUM accumulation as deg[j] * bias[f]
            # (a K=1 matmul); the recip scale below then yields
            # h@W.T + bias exactly, skipping a DVE add per window.
            nc.tensor.matmul(out=out_p[:],
                             lhsT=deg_t[:, w * P:(w + 1) * P],
                             rhs=bias_t[:],
                             start=False, stop=True)
            out_s = epool.tile([P, out_f], f32, tag="outs", name=f"outs{w}")
            nc.scalar.activation(out=out_s[:], in_=out_p[:],
                                 func=mybir.ActivationFunctionType.Copy,
                                 scale=recip_t[:, w:w + 1])
            # out-stores ride the Scalar HWDGE queue: the Sync queue stays a
            # pure idx-slice pipe, so no mid-stream store dispatch can delay
            # a slice a gather is waiting on
            nc.scalar.dma_start(out=out_d[w * P:(w + 1) * P, :], in_=out_s[:])

    nc.compile()
    return nc


def make_in_maps(plan, x, W, b):
    in_f = x.shape[1]
    out_f = W.shape[0]
    import ml_dtypes
    xb = np.ascontiguousarray(x, dtype=np.float32).astype(ml_dtypes.bfloat16)
    base = {
        "xb": xb,
        "wt": np.ascontiguousarray(W.T).astype(ml_dtypes.bfloat16),
        "bias": np.asarray(b, np.float32)[None, :].astype(ml_dtypes.bfloat16),
        "iota": np.tile(np.arange(P, dtype=np.float32)[None, :],
                        (P, 1)).astype(ml_dtypes.bfloat16),
    }
    in_maps = []
    for k in range(plan.n_cores):
        m = dict(base)
        m["recip"] = plan.core_recip[k]
        m["deg"] = plan.core_deg[k]
        for name, arr in plan.core_arrays[k].items():
            s = name.split("_")[1]
            if plan.Csum[s] == 0:
                continue
            m[name] = arr
        in_maps.append(m)
    return in_maps


def run(x, edge_index, n_nodes, W, b, trace=False, trace_cores=None):
    from concourse.bass_utils import run_bass_kernel_spmd

    x = np.asarray(x)
    edge_index = np.asarray(edge_index)
    W = np.asarray(W)
    b = np.asarray(b)
    n_nodes = int(n_nodes)
    row = edge_index[0].astype(np.int64)
    col = edge_index[1].astype(np.int64)

    plan = Plan(row, col, n_nodes)
    nc = build_program(plan, x.shape[1], W.shape[0])
    in_maps = make_in_maps(plan, x, W, b)
    res = run_bass_kernel_spmd(nc, in_maps, core_ids=list(range(plan.n_cores)),
                               trace=trace, trace_cores=trace_cores)
    out = np.concatenate(
        [res.results[k]["out"][:plan.d_core] for k in range(plan.n_cores)],
        axis=0)
    return np.ascontiguousarray(out, dtype=np.float32), res


def kernel(x, edge_index, n_nodes, W, b):
    out, _ = run(x, edge_index, n_nodes, W, b)
    return out


# revision 20
# speedup vs baseline: 1.2137x; 1.2137x over previous
"""GCN layer (gather + segment-sum + degree-normalize + linear) on 8 Trainium2 cores.

Strategy
--------
Destination-node sharding: core k owns dest rows [k*D, (k+1)*D), D = n_nodes/8.
The host groups each core's edges by 128-dest windows (dest-sorted); the
on-device segment-sum is done per 128-edge chunk with a PE matmul
(lhsT = gathered source features G [128 edge, 128 feat] bf16, rhs = selection
matrix S [128 edge, 128 dest] fp8 with S[e, j] = (col_rel[e] == j)),
accumulating aggT[feat, dest] in PSUM per window. aggT is copied to SBUF
(Scalar) and used directly as lhsT of the linear matmul; bias is folded into
that matmul's PSUM accumulation as a K=1 matmul of deg[j] x bias[f], so the
Scalar PSUM->SBUF copy with a per-partition 1/max(deg,1) scale yields
h @ W.T + b exactly with no DVE add. S is built on DVE in 32-chunk batches
(bf16 crel/iota inputs, fp8 output). No collectives; each core writes its
own output slice and the host concatenates.

The gather stream is the critical path and is DESC-GEN-BOUND: SWDGE Q7
ucode generates ~1 descriptor per 8.0ns per queue solo, degrading to
~9.1ns/desc under SBUF-port contention (G-tile DMA writes, DVE S-builds,
PE reads share ports with the Q7s' idx reads + ring writes); 4 queues run
concurrently for ~2.3ns/desc aggregate over ~108.5K descriptors/core.
Descriptor execution on the 16 DMA engines is NOT the wall (engines ~45%
busy). Every contention cut matters: fp8 S halves the S-build write and PE
read traffic (S is 0/1, exact in fp8; mixed bf16 lhsT x fp8 rhs matmul is
accepted), and the bias fold removes 49 DVE adds.

Schedule: one dma_gather call per 16-chunk (2048-edge) G tile, 16 tiles in
flight per stream; call->queue assignment greedily balances per-queue
descriptor totals (the stream ends when the most-loaded queue finishes) with
a local swap pass; the stream tail is split into 8/8/6/4-chunk calls so the
last wave's post-stream matmul drain is short. gidx loads are split into
2-call (~131KB) slices on the Sync HWDGE queue -- big enough to stream at
full HWDGE rate (small transfers are ~600ns dispatch-bound), small enough
that a call gates only on its own slice; all slices land in the first ~25us.
No warm-up gathers: the ~12us cold start is the one-time global
extended-ucode LOAD_LIB, paid before the first call regardless (first real
gather dispatches at ~17.5us; per-queue first-call cost after the lib load
is ~100ns).

dma_gather facts (measured): idx arrays are int16, wrapped [16, N/16] and
replicated into all eight 16-partition groups; single_packet=False is
required for calls over 1024 indices. int16 limits a gather call's index
range to 32768 rows, so edges are split into lo/hi source streams gathered
from base x[0] / x[32768]. x is gathered in bf16 (256B/row, the descriptor
minimum); rel err ~2.5e-3, well inside the 2e-2 gate. Do NOT mark pad
slots with negative indices to skip their descriptors: the NX-side ring
reservation sizes from num_idxs_reg while the Q7 trims by scanning, and the
mismatch corrupts the descriptor rings (hangs) unless the register carries
the exact per-core post-trim count.

Measured: 280us vs 298us for the 2-lane 32-chunk-block baseline; floor for
this structure is ~247us (solo-rate desc-gen 219us + 17.5us lib load +
~10us drain).
"""
import sys
sys.path.insert(0, "/opt/trn_rl_repo")

import numpy as np

P = 128
GATHER_SPLIT = 32768       # max rows addressable by a signed-int16 gather index
CALL_CHUNKS = 16           # standard gather call / G tile size in 128-edge chunks
SLICE_CALLS = 2            # gidx calls per DMA slice (~131KB each)
SBATCH = 32                # S-matrix build batch, in chunks
GBUFS = 16                 # in-flight G tiles per stream (~4 per queue)
N_CORES = 8


def _ceil_div(a, b):
    return -(-a // b)


def _wrap_idx(ix):
    """[N] int16 -> [128, N/16], idx i at [i%16, i//16], replicated into the
    eight 16-partition groups (the tx/rx Q7 cpus of every SWDGE queue each
    read their own group)."""
    n = len(ix)
    assert n % 16 == 0
    w = np.zeros((P, n // 16), np.int16)
    blk = ix.reshape(-1, 16).T
    for g in range(8):
        w[16 * g:16 * (g + 1), :] = blk
    return w


class Plan:
    """Host-side sharding: per-core per-stream edge arrays with a chunk
    structure (windows x chunk counts) identical across cores, so a single
    SPMD program serves all cores."""

    def __init__(self, row, col, n_nodes, n_cores=N_CORES,
                 call_chunks=CALL_CHUNKS, gather_split=GATHER_SPLIT):
        assert n_nodes % n_cores == 0
        self.n_cores = n_cores
        self.n_nodes = n_nodes
        self.d_core = n_nodes // n_cores
        self.n_win = _ceil_div(self.d_core, P)
        self.call_chunks = call_chunks
        self.gather_split = gather_split

        order = np.argsort(col, kind="stable")
        rs = row[order]
        cs = col[order]
        bounds = np.searchsorted(cs, np.arange(n_cores + 1) * self.d_core)

        # in-degree (clamped to 1) per node, laid out per core as
        # [P, n_win] f32 reciprocal: recip[j, w] = 1/deg of dest w*128+j
        deg = np.bincount(cs, minlength=n_nodes).astype(np.float32)
        deg = np.maximum(deg, 1.0)
        recip = (1.0 / deg)
        pad = self.n_win * P - self.d_core
        self.core_recip = []
        self.core_deg = []
        import ml_dtypes
        for k in range(n_cores):
            r = recip[k * self.d_core:(k + 1) * self.d_core]
            r = np.concatenate([r, np.zeros(pad, np.float32)])
            self.core_recip.append(
                np.ascontiguousarray(r.reshape(self.n_win, P).T))
            dg = deg[k * self.d_core:(k + 1) * self.d_core]
            dg = np.concatenate([dg, np.zeros(pad, np.float32)])
            self.core_deg.append(
                dg[None, :].astype(ml_dtypes.bfloat16))

        W = self.n_win
        per_core = []  # [k][stream] = (rows, local_cols, per-window counts)
        cnt = {"lo": np.zeros(W, np.int64), "hi": np.zeros(W, np.int64)}
        for k in range(n_cores):
            a, b = bounds[k], bounds[k + 1]
            r_k = rs[a:b]
            lc_k = cs[a:b] - k * self.d_core
            lo = r_k < gather_split
            streams = {}
            for sname, mask in (("lo", lo), ("hi", ~lo)):
                r_s = r_k[mask]
                lc_s = lc_k[mask]
                counts = np.bincount(lc_s >> 7, minlength=W)
                streams[sname] = (r_s, lc_s, counts)
                cnt[sname] = np.maximum(cnt[sname], -(-counts // P))
            per_core.append(streams)
        cnt["lo"] = np.maximum(cnt["lo"], 1)  # every window gets >=1 chunk
        self.cnt = cnt
        self.off = {s: np.concatenate([[0], np.cumsum(cnt[s])]) for s in cnt}
        self.Csum = {s: int(self.off[s][-1]) for s in cnt}
        # call partition per stream: standard call_chunks-sized calls, with
        # the tail split into small calls (the last wave's calls land nearly
        # together, so the post-stream matmul drain is one tail-call deep)
        self.call_bounds = {}
        for s in cnt:
            c = self.Csum[s]
            bounds = [0]
            tail = [4, 6, 8, 8]  # last calls, smallest last
            while c - bounds[-1] > sum(tail):
                bounds.append(bounds[-1] + call_chunks)
            rem = c - bounds[-1]
            for t in reversed(tail):
                if rem <= 0:
                    break
                take = min(t, rem)
                bounds.append(bounds[-1] + take)
                rem -= take
            self.call_bounds[s] = np.array(bounds)
        self.NB = {s: len(self.call_bounds[s]) - 1 for s in cnt}
        self.Npad = {s: _ceil_div(self.Csum[s], call_chunks)
                     * call_chunks * P for s in cnt}

        self.core_arrays = []
        import ml_dtypes
        for k in range(n_cores):
            arrs = {}
            for sname in ("lo", "hi"):
                r_s, lc_s, counts = per_core[k][sname]
                off = self.off[sname]
                base = 0 if sname == "lo" else gather_split
                gidx = np.zeros(self.Npad[sname], np.int16)
                crel = np.full(self.Csum[sname] * P, -1, np.int8)
                if len(r_s):
                    starts = np.concatenate([[0], np.cumsum(counts)])
                    adj = off[:-1] * P - starts[:-1]
                    dst = np.arange(len(r_s)) + adj[lc_s >> 7]
                    gidx[dst] = (r_s - base).astype(np.int16)
                    crel[dst] = (lc_s & 127).astype(np.int8)
                arrs[f"gidx_{sname}"] = _wrap_idx(gidx)
                # [P edge-slot, Csum chunk] bf16 (values -1..127, exact)
                arrs[f"crel_{sname}"] = np.ascontiguousarray(
                    crel.reshape(self.Csum[sname], P).T.astype(np.float32)
                ).astype(ml_dtypes.bfloat16)
            self.core_arrays.append(arrs)

    @property
    def total_chunks(self):
        return self.Csum["lo"] + self.Csum["hi"]


def _patch_swdge_lane_by_queue():
    """Pin each dma_gather's DMASW semaphore lane to its SWDGE queue number.

    Tile assigns DMASW lanes round-robin in scheduled order, which breaks when
    instructions on different queues (whose completions are only FIFO within a
    queue) share a lane. Two lanes per queue keep per-lane completion in-order
    and let a call's desc-gen overlap the previous same-queue call's DMA
    completion.
    """
    import concourse.tile_sem_assignment as tsa
    from concourse import mybir
    if getattr(tsa.TileClockTick, "_lane_by_queue_patch", False):
        return
    orig = tsa.TileClockTick._assign_tick

    def patched(self, inst):
        if isinstance(inst, mybir.InstDMAGatherAnt):
            if not hasattr(self, "_q_lane_ctr"):
                self._q_lane_ctr = {}
            q = inst.queue_num
            n = self._q_lane_ctr.get(q, 0)
            self._q_lane_ctr[q] = n + 1
            saved = self.next_sw_dma_idx
            self.next_sw_dma_idx = q * 2 + (n % 2)
            try:
                return orig(self, inst)
            finally:
                self.next_sw_dma_idx = saved
        return orig(self, inst)

    tsa.TileClockTick._assign_tick = patched
    tsa.TileClockTick._lane_by_queue_patch = True


def build_program(plan, in_f, out_f):
    """Emit the SPMD Bass program (shared by all cores)."""
    from concourse import bacc, mybir
    import concourse.tile as tile
    from contextlib import ExitStack

    _patch_swdge_lane_by_queue()

    f32 = mybir.dt.float32
    i16 = mybir.dt.int16
    bf16 = mybir.dt.bfloat16
    fp8 = mybir.dt.float8e4

    W = plan.n_win
    CC = plan.call_chunks

    nc = bacc.Bacc("TRN2", target_bir_lowering=False, debug=False,
                   num_devices=plan.n_cores, num_swdge_queues=4)

    x_d = nc.dram_tensor("xb", [plan.n_nodes, in_f], bf16,
                         kind="ExternalInput")
    wt_d = nc.dram_tensor("wt", [in_f, out_f], bf16, kind="ExternalInput")
    bias_d = nc.dram_tensor("bias", [1, out_f], bf16, kind="ExternalInput")
    deg_d = nc.dram_tensor("deg", [1, W * P], bf16, kind="ExternalInput")
    iota_d = nc.dram_tensor("iota", [P, P], bf16, kind="ExternalInput")
    recip_d = nc.dram_tensor("recip", [P, W], f32, kind="ExternalInput")
    gidx_d, crel_d = {}, {}
    for s in ("lo", "hi"):
        if plan.Csum[s] == 0:
            continue
        gidx_d[s] = nc.dram_tensor(f"gidx_{s}", [P, plan.Npad[s] // 16], i16,
                                   kind="ExternalInput")
        crel_d[s] = nc.dram_tensor(f"crel_{s}", [P, plan.Csum[s]], bf16,
                                   kind="ExternalInput")
    out_d = nc.dram_tensor("out", [W * P, out_f], f32, kind="ExternalOutput")

    x_base = {"lo": x_d[:], "hi": x_d[plan.gather_split:, :]}

    # gather calls in consumption order, interleaved by stream progress;
    # greedy min-load queue assignment (in descriptors) so all 4 queues'
    # desc-gen streams finish together
    def _frac(s, b):
        cb = plan.call_bounds[s]
        return (cb[b] + cb[b + 1]) / 2 / plan.Csum[s]
    call_order = sorted(
        [(s, b) for s in ("lo", "hi") for b in range(plan.NB[s])],
        key=lambda sb: _frac(*sb))
    def _size(sb):
        s, b = sb
        return int(plan.call_bounds[s][b + 1] - plan.call_bounds[s][b])

    qload = [0, 0, 0, 0]
    call_queue = {}
    for s, b in call_order:
        q = min(range(4), key=lambda i: (qload[i], i))
        call_queue[(s, b)] = q
        qload[q] += _size((s, b))
    # local swap pass: equalize per-queue totals (gen-rate-bound stream ends
    # when the most-loaded queue finishes) without moving calls far from
    # their consumption position
    for _ in range(64):
        hi_q = max(range(4), key=lambda i: qload[i])
        lo_q = min(range(4), key=lambda i: qload[i])
        gap = qload[hi_q] - qload[lo_q]
        if gap < 4:
            break
        best = None
        for i, sb in enumerate(call_order):
            if call_queue[sb] != hi_q:
                continue
            for j in range(max(0, i - 4), min(len(call_order), i + 5)):
                sb2 = call_order[j]
                if call_queue[sb2] != lo_q:
                    continue
                d = _size(sb) - _size(sb2)
                if 0 < d <= gap:
                    if best is None or d > best[0]:
                        best = (d, sb, sb2)
        if best is None:
            break
        _, sb, sb2 = best
        call_queue[sb], call_queue[sb2] = lo_q, hi_q
        qload[hi_q] -= best[0]
        qload[lo_q] += best[0]

    with tile.TileContext(nc) as tc, ExitStack() as ctx:
        cpool = ctx.enter_context(tc.tile_pool(name="const", bufs=1))
        gpool = {s: ctx.enter_context(tc.tile_pool(name=f"g_{s}", bufs=GBUFS))
                 for s in ("lo", "hi")}
        spool = {s: ctx.enter_context(tc.tile_pool(name=f"s_{s}", bufs=3))
                 for s in ("lo", "hi")}
        epool = ctx.enter_context(tc.tile_pool(name="epi", bufs=3))
        apool = ctx.enter_context(tc.tile_pool(name="psum_a", bufs=4,
                                               space="PSUM"))
        hpool = ctx.enter_context(tc.tile_pool(name="psum_h", bufs=2,
                                               space="PSUM"))

        # ---- gidx slices: SLICE_CALLS calls each (~131KB), interleaved
        # lo/hi so both streams' first calls gate only on the first slices.
        # All on the Sync HWDGE queue, emitted upfront: big transfers stream
        # at full rate and land before the gather stream needs the engines.
        # slice j covers calls [j*SLICE_CALLS, (j+1)*SLICE_CALLS)
        idx_slices = {}
        slice_bounds = {}
        for s in ("lo", "hi"):
            cb = plan.call_bounds[s]
            sb_ = [int(cb[min(j, plan.NB[s])])
                   for j in range(0, plan.NB[s] + SLICE_CALLS, SLICE_CALLS)]
            slice_bounds[s] = sb_
        slice_order = sorted(
            [(s, j) for s in ("lo", "hi")
             for j in range(len(slice_bounds[s]) - 1)],
            key=lambda sj:
            (slice_bounds[sj[0]][sj[1]] + slice_bounds[sj[0]][sj[1] + 1])
            / 2 / plan.Csum[sj[0]])
        for s, j in slice_order:
            a = slice_bounds[s][j] * P // 16
            z = slice_bounds[s][j + 1] * P // 16
            if z == a:
                continue
            it = cpool.tile([P, z - a], i16, name=f"I{s}{j}")
            nc.sync.dma_start(out=it[:], in_=gidx_d[s][:, a:z])
            idx_slices[(s, j)] = it

        # ---- constants (gate only S builds and the per-window epilogue) ----
        iota_t = cpool.tile([P, P], bf16)
        nc.scalar.dma_start(out=iota_t[:], in_=iota_d[:])
        crel_t = {}
        for s in ("lo", "hi"):
            if plan.Csum[s] == 0:
                continue
            cri = cpool.tile([P, plan.Csum[s]], bf16, name=f"crel{s}")
            nc.scalar.dma_start(out=cri[:], in_=crel_d[s][:])
            crel_t[s] = cri
        wt_t = cpool.tile([in_f, out_f], bf16)
        nc.scalar.dma_start(out=wt_t[:], in_=wt_d[:])
        bias_t = cpool.tile([1, out_f], bf16)
        nc.scalar.dma_start(out=bias_t[:], in_=bias_d[:])
        deg_t = cpool.tile([1, W * P], bf16)
        nc.scalar.dma_start(out=deg_t[:], in_=deg_d[:])
        recip_t = cpool.tile([P, W], f32)
        nc.scalar.dma_start(out=recip_t[:], in_=recip_d[:])

        # ---- gather calls: one per 16-chunk G tile ----
        g_tiles = {}

        def get_g(s, b):
            if (s, b) not in g_tiles:
                cb = plan.call_bounds[s]
                c0, c1 = int(cb[b]), int(cb[b + 1])
                nch = c1 - c0
                gt = gpool[s].tile([P, CC * in_f], bf16, name=f"G{s}{b}",
                                   tag=f"G{s}")
                j = b // SLICE_CALLS
                it = idx_slices[(s, j)]
                off = (c0 - slice_bounds[s][j]) * P // 16
                nc.gpsimd.dma_gather(
                    gt[:, :nch * in_f]
                    .rearrange("p (c e) -> p c e", e=in_f),
                    x_base[s],
                    it[:, off:off + nch * P // 16],
                    nch * P,
                    nch * P,
                    in_f,
                    single_packet=False,
                    queue_num=call_queue[(s, b)],
                )
                g_tiles[(s, b)] = gt
            return g_tiles[(s, b)]

        # ---- lazily-emitted batched S builds (one DVE tensor_tensor
        # is_equal per SBATCH chunks, bf16 in/out) ----
        s_tiles = {}

        def get_s(s, sb):
            if (s, sb) not in s_tiles:
                st = spool[s].tile([P, SBATCH * P], fp8, name=f"S{s}{sb}",
                                   tag=f"S{s}")
                nb = min(SBATCH, plan.Csum[s] - sb * SBATCH)
                in0 = crel_t[s][:, sb * SBATCH:sb * SBATCH + nb] \
                    .to_broadcast([P, nb, P])
                in1 = iota_t[:][:, None, :].to_broadcast([P, nb, P])
                outv = st[:].rearrange("p (b j) -> p b j", j=P)[:, :nb, :]
                nc.vector.tensor_tensor(out=outv, in0=in0, in1=in1,
                                        op=mybir.AluOpType.is_equal)
                s_tiles[(s, sb)] = st
            return s_tiles[(s, sb)]

        # ---- pre-issue every gather call so the Pool engine always has
        # ready calls on all 4 queues ----
        for s, b in call_order:
            get_g(s, b)

        # ---- main window loop ----
        for w in range(W):
            chunks = []
            for s in ("lo", "hi"):
                chunks += [(s, c) for c in
                           range(plan.off[s][w], plan.off[s][w + 1])]
            psum_aggT = apool.tile([P, in_f], f32, tag="aggT",
                                   name=f"aggT{w}")
            n = len(chunks)
            for i, (s, c) in enumerate(chunks):
                b = int(np.searchsorted(plan.call_bounds[s], c,
                                        side="right")) - 1
                slot = c - int(plan.call_bounds[s][b])
                sb, ssub = divmod(c, SBATCH)
                gt = get_g(s, b)
                st = get_s(s, sb)
                nc.tensor.matmul(
                    out=psum_aggT[:],
                    lhsT=gt[:, slot * in_f:(slot + 1) * in_f],
                    rhs=st[:, ssub * P:(ssub + 1) * P],
                    start=(i == 0), stop=(i == n - 1))

            hT_t = epool.tile([P, P], bf16, tag="hT", name=f"hT{w}")
            nc.scalar.activation(out=hT_t[:], in_=psum_aggT[:],
                                 func=mybir.ActivationFunctionType.Copy)
            out_p = hpool.tile([P, out_f], f32, tag="outp", name=f"outp{w}")
            nc.tensor.matmul(out=out_p[:], lhsT=hT_t[:], rhs=wt_t[:],
                             start=True, stop=False)
            # bias folded into the PSUM accumulation as deg[j] * bias[f]
            # (a K=1 matmul); the recip scale below then yields
            # h@W.T + bias exactly, skipping a DVE add per window.
            nc.tensor.matmul(out=out_p[:],
                             lhsT=deg_t[:, w * P:(w + 1) * P],
                             rhs=bias_t[:],
                             start=False, stop=True)
            out_s = epool.tile([P, out_f], f32, tag="outs", name=f"outs{w}")
            nc.scalar.activation(out=out_s[:], in_=out_p[:],
                                 func=mybir.ActivationFunctionType.Copy,
                                 scale=recip_t[:, w:w + 1])
            nc.sync.dma_start(out=out_d[w * P:(w + 1) * P, :], in_=out_s[:])

    nc.compile()
    return nc


def make_in_maps(plan, x, W, b):
    in_f = x.shape[1]
    out_f = W.shape[0]
    import ml_dtypes
    xb = np.ascontiguousarray(x, dtype=np.float32).astype(ml_dtypes.bfloat16)
    base = {
        "xb": xb,
        "wt": np.ascontiguousarray(W.T).astype(ml_dtypes.bfloat16),
        "bias": np.asarray(b, np.float32)[None, :].astype(ml_dtypes.bfloat16),
        "iota": np.tile(np.arange(P, dtype=np.float32)[None, :],
                        (P, 1)).astype(ml_dtypes.bfloat16),
    }
    in_maps = []
    for k in range(plan.n_cores):
        m = dict(base)
        m["recip"] = plan.core_recip[k]
        m["deg"] = plan.core_deg[k]
        for name, arr in plan.core_arrays[k].items():
            s = name.split("_")[1]
            if plan.Csum[s] == 0:
                continue
            m[name] = arr
        in_maps.append(m)
    return in_maps


def run(x, edge_index, n_nodes, W, b, trace=False, trace_cores=None):
    from concourse.bass_utils import run_bass_kernel_spmd

    x = np.asarray(x)
    edge_index = np.asarray(edge_index)
    W = np.asarray(W)
    b = np.asarray(b)
    n_nodes = int(n_nodes)
    row = edge_index[0].astype(np.int64)
    col = edge_index[1].astype(np.int64)

    plan = Plan(row, col, n_nodes)
    nc = build_program(plan, x.shape[1], W.shape[0])
    in_maps = make_in_maps(plan, x, W, b)
    res = run_bass_kernel_spmd(nc, in_maps, core_ids=list(range(plan.n_cores)),
                               trace=trace, trace_cores=trace_cores)
    out = np.concatenate(
        [res.results[k]["out"][:plan.d_core] for k in range(plan.n_cores)],
        axis=0)
    return np.ascontiguousarray(out, dtype=np.float32), res


def kernel(x, edge_index, n_nodes, W, b):
    out, _ = run(x, edge_index, n_nodes, W, b)
    return out
